# revision 1
# baseline (speedup 1.0000x reference)
"""Trainium2 Bass kernel: single-head attention (B=4, S=2048, D=1024) on 8 NeuronCores.

Sharding: data-parallel over (batch, query-half): core c handles batch c//2,
query rows [c%2*1024, (c%2+1)*1024), and (dist mode) computes the K-projection
only for its own kv half; the pair {2b, 2b+1} exchanges K^T via a 2-rank
AllGather overlapped with the Q-projection and the local-half attention.

Math per core (all matmuls bf16, fp32 PSUM accumulation):
  QT[dk,q]  = Wq(lhsT) . XqT(rhs)                 (+bq)
  KT[dk,s]  = Wk(lhsT) . XkvT(rhs)                (+bk)   [local half, AG for rest]
  sT[s,q]   = KT-tiles(lhsT) . QT(rhs)            scores^T
  eT[s,q]   = exp(sT / sqrt(dk) [+ maskT])        ScalarE, PSUM->SBUF bf16
  sums[1,q] = ones(lhsT) . eT(rhs)                softmax denominators
  HT[dm,q]  = Xkv-tiles(lhsT) . eT(rhs)           H = probs_unnorm @ Xkv
  out[q,dv] = HT-tiles(lhsT) . Wv(rhs)  (+bv)     (probs@Xkv)@Wv == probs@(Xkv@Wv)
  out      *= 1/sums  (per-partition scale on ScalarE, fused with PSUM->SBUF copy)

DMA design (from trace analysis):
- Each hw queue serves ~one dma_start at a time with multi-us turnaround, so
  aggregate startup bandwidth scales with in-flight dma_starts: keep payloads
  ~128-256KB and fan out across all ring slots of all 3 DMA-capable engines.
- A dma_start's wait condition blocks the ISSUING ENGINE's stream, so: the
  scalar engine (which runs every PSUM->SBUF activation) gets only a handful
  of pre-activation issues; collective readbacks (which wait on the AllGather)
  go on sync+gpsimd after everything else those engines must issue first.
- wq/wk are host-packed d_key-block-major, xqt/xkvt chunk-tile-major, and xkv
  (j, s)-tile-major, so every DMA has >=2KB contiguous segments and each PE
  accumulation group depends on the minimum payload (first matmul needs only
  ~0.5MB: wk block 0 + the leading 128-col xkvt chunk).

dist mode details: the s axis lives in LOCAL-relative order on each core
(m-tiles 0..7 = own kv half, 8..15 = partner's). The host feeds xkv/maskt
pre-permuted to match. The AllGather output is rank-ordered (identical layout
on both cores), so the partner block is recovered SPMD-uniformly via the exact
bf16-bit identity  remote = g0 XOR g1 XOR local  on uint32 views (own block
round-trips bit-identically through the collective).
"""

import os
import numpy as np
import ml_dtypes

B, S, D = 4, 2048, 1024
N_CORES = 8
QL = S // 2  # query rows per core (1024)
BF16 = ml_dtypes.bfloat16

_cache: dict = {}


def _kch(kvl):
    # K-projection chunk widths: narrow leading chunks shrink the
    # startup-critical DMA payload
    lead = int(os.environ.get("KERNEL_KCH", "256"))
    ch = [512] if lead == 512 else [lead, lead]
    while sum(ch) < kvl:
        ch.append(512)
    return ch


def _build(dist: bool, with_mask: bool, with_bq: bool, with_bk: bool,
           with_bv: bool, ps_bufs: int = 4):
    import concourse.bass as bass
    import concourse.mybir as mybir
    import concourse.tile as tile
    from concourse import bacc

    fp32 = mybir.dt.float32
    bf16 = mybir.dt.bfloat16
    uint32 = mybir.dt.uint32

    nc = bacc.Bacc("TRN2", target_bir_lowering=False, debug=False,
                   num_devices=N_CORES)

    KVL = QL if dist else S  # kv rows projected locally
    NT_D = D // 128    # 8 tiles along d_model / d_key
    NT_S = S // 128    # 16 tiles along s
    NT_L = NT_S // 2   # 8 (local-half s tiles in dist mode)
    NQ = QL // 512     # 2 query chunks of 512
    NV = D // 512      # 2 dv chunks of 512
    SCALE = 1.0 / float(np.sqrt(D))

    KCH = _kch(KVL)
    KOFF = [sum(KCH[:c]) for c in range(len(KCH))]
    LAST_C = len(KCH) - 1

    # all activations host-packed into SBUF tile layouts (fully-contiguous
    # DMA segments); wq/wk packed d_key-block-major
    xqt_d = nc.dram_tensor("xqt", (128, NT_D * QL), bf16, kind="ExternalInput")
    xkvt_d = nc.dram_tensor("xkvt", (128, NT_D * KVL), bf16,
                            kind="ExternalInput")
    xkv_d = nc.dram_tensor("xkv", (128, NT_D * S), bf16, kind="ExternalInput")
    wq_d = nc.dram_tensor("wq", (128, NT_D * D), bf16, kind="ExternalInput")
    wk_d = nc.dram_tensor("wk", (128, NT_D * D), bf16, kind="ExternalInput")
    wv_d = nc.dram_tensor("wv", (D, D), bf16, kind="ExternalInput")
    if with_bq:
        bq_d = nc.dram_tensor("bq", (128, 8), fp32, kind="ExternalInput")
    if with_bk:
        bk_d = nc.dram_tensor("bk", (128, 8), fp32, kind="ExternalInput")
    if with_bv:
        bv_d = nc.dram_tensor("bv", (1, D), bf16, kind="ExternalInput")
    if with_mask:
        maskt_d = nc.dram_tensor("maskt", (S, QL), bf16, kind="ExternalInput")
    out_d = nc.dram_tensor("out", (QL, D), bf16, kind="ExternalOutput")

    NSPLIT = 4  # K^T exchange split into NSPLIT AllGathers so the first
    # one triggers as soon as the first d_key rows land — the whole
    # collective+readback pipeline finishes ~30us earlier than a single AG
    IPG = NT_D // NSPLIT  # i-blocks per AG
    if dist:
        cc_ins = [nc.dram_tensor(f"cc_in{h}", (IPG * 128, QL), bf16)
                  for h in range(NSPLIT)]
        cc_outs = [nc.dram_tensor(f"cc_out{h}", (2 * IPG * 128, QL), bf16)
                   for h in range(NSPLIT)]
        groups = [[2 * g, 2 * g + 1] for g in range(4)]

    with tile.TileContext(nc) as tc:
        with (
            tc.tile_pool(name="cons", bufs=1) as cons,
            tc.tile_pool(name="wkb", bufs=1) as wkbp,
            tc.tile_pool(name="wqb", bufs=1) as wqbp,
            tc.tile_pool(name="big", bufs=3) as bigp,
            tc.tile_pool(name="kt", bufs=1) as ktp,
            tc.tile_pool(name="et", bufs=1) as etp,
            tc.tile_pool(name="xk", bufs=1) as xkp,
            tc.tile_pool(name="xq", bufs=2) as xqp,
            tc.tile_pool(name="xkv", bufs=4) as xkvp,
            tc.tile_pool(name="gch", bufs=4) as gchp,
            tc.tile_pool(name="outp", bufs=2) as outp,
            tc.tile_pool(name="mask", bufs=2) as maskp,
            tc.tile_pool(name="ps", bufs=ps_bufs,
                         space=bass.MemorySpace.PSUM) as psp,
            tc.tile_pool(name="pss", bufs=2, space=bass.MemorySpace.PSUM) as pssp,
            tc.tile_pool(name="pst", bufs=1, space=bass.MemorySpace.PSUM) as pstp,
        ):
            # ---- startup-critical DMA wave ----
            wk_blks = [wkbp.tile([128, D], bf16, tag=f"wkb{i}",
                                 name=f"wkb{i}")
                       for i in range(NT_D)]
            xk_chs = [xkp.tile([128, NT_D, KCH[c]], bf16,
                               tag=f"xk{c}", name=f"xk{c}")
                      for c in range(len(KCH))]

            def ld_wk(eng, i):   # one i-block: 256KB
                eng.dma_start(wk_blks[i][:],
                              wk_d.ap()[:, i * D:(i + 1) * D])

            def ld_xk(eng, c, part, nparts):  # 1/nparts of chunk c
                w = KCH[c]
                jn = NT_D // nparts
                base = NT_D * KOFF[c]
                eng.dma_start(
                    xk_chs[c][:, part * jn:(part + 1) * jn, :],
                    xkvt_d.ap()[:, base + part * jn * w:
                                base + (part + 1) * jn * w])

            sy, sc, gp = nc.sync, nc.scalar, nc.gpsimd
            # (engine, payload) in consumer-priority order. ~256-512KB per
            # dma_start: few enough issue ops (~0.65us each) that the whole
            # 4MB critical set is in flight by ~12us, big enough to keep the
            # queue ring slots saturated. scalar gets only 2 issues so its
            # first K activation isn't delayed.
            # the first group's j=0 operands ship as tiny dedicated pieces:
            # cold rings run at ~100GB/s, so 32-64KB puts the first matmul
            # ~2us earlier; the rest of wk0/chunk0 follows right behind
            sy.dma_start(wk_blks[0][:, 0:128], wk_d.ap()[:, 0:128])
            gp.dma_start(xk_chs[0][:, 0:1, :],
                         xkvt_d.ap()[:, 0:KCH[0]])
            sc.dma_start(wk_blks[0][:, 128:512], wk_d.ap()[:, 128:512])
            sy.dma_start(wk_blks[0][:, 512:D], wk_d.ap()[:, 512:D])
            sc.dma_start(xk_chs[0][:, 1:4, :],
                         xkvt_d.ap()[:, KCH[0]:4 * KCH[0]])
            ld_xk(gp, 0, 1, 2)
            ld_wk(sy, 1); ld_wk(sc, 2); ld_wk(gp, 3)
            ld_xk(sy, 1, 0, 2); ld_xk(gp, 1, 1, 2)
            # with the NSPLIT-way K loop, chunk 2 is consumed ~3.4us after
            # the first group — it outranks wk4-7 (needed 14-20us later)
            if len(KCH) > 2:
                ld_xk(sy, 2, 0, 2); ld_xk(gp, 2, 1, 2)
            ld_wk(sy, 4); ld_wk(gp, 5)
            ld_wk(sy, 6); ld_wk(gp, 7)
            for c in range(3, len(KCH)):
                ld_xk(sy, c, 0, 2); ld_xk(gp, c, 1, 2)

            # ---- Q-stage loads (needed ~28us later): sync + gpsimd ----
            wq_blks = [wqbp.tile([128, D], bf16, tag=f"wqb{i}",
                                 name=f"wqb{i}")
                       for i in range(NT_D)]
            xq_chs = [xqp.tile([128, NT_D, 512], bf16, tag="xq",
                               name=f"xq{n}")
                      for n in range(NQ)]

            def ld_xq(eng, n, part):
                eng.dma_start(
                    xq_chs[n][:, part * 4:(part + 1) * 4, :],
                    xqt_d.ap()[:, n * NT_D * 512 + part * 2048:
                               n * NT_D * 512 + (part + 1) * 2048])

            ld_xq(sy, 0, 0); ld_xq(gp, 0, 1)
            for i in range(NT_D):
                (sy if i % 2 == 0 else gp).dma_start(
                    wq_blks[i][:], wq_d.ap()[:, i * D:(i + 1) * D])
            # xq chunk 1 is needed only at the second Q-proj pass (~54us);
            # it is issued after the first K quarter instead (see below) so
            # its 1MB doesn't contend with the startup-critical wire

            # wv on gpsimd BEFORE the collective trigger (which blocks the
            # gpsimd stream until the cc_in stores land)
            wv_sb = bigp.tile([128, NT_D, D], bf16, tag="big", name="wv_sb")
            for h in range(4):
                nc.gpsimd.dma_start(
                    wv_sb[:, 2 * h:2 * h + 2, :],
                    wv_d.ap()[2 * h * 128:(2 * h + 2) * 128, :]
                    .rearrange("(j p) d -> p j d", p=128))

            # ---- constants / biases (tiny, off the critical path) ----
            ones_col = cons.tile([128, 1], fp32, tag="ones_col")
            nc.gpsimd.memset(ones_col[:], 1.0)
            ident1 = cons.tile([1, 1], fp32, tag="ident1")
            nc.gpsimd.memset(ident1[:], 1.0)
            if with_bv:
                bv_sb = cons.tile([1, D], bf16, tag="bv")
                nc.sync.dma_start(bv_sb[:], bv_d.ap()[:])
            if with_bq:
                bq_sb = cons.tile([128, 8], fp32, tag="bq")
                nc.sync.dma_start(bq_sb[:], bq_d.ap()[:])
            if with_bk:
                bk_sb = cons.tile([128, 8], fp32, tag="bk")
                nc.sync.dma_start(bk_sb[:], bk_d.ap()[:])

            kt_sb = ktp.tile([128, NT_D, KVL], bf16, tag="kt")
            if dist:
                ktr_sb = ktp.tile([128, NT_D, QL], bf16, tag="ktr")
            et_sb = etp.tile([128, NT_S, QL], bf16, tag="et")

            # xkv prefetch helper (gpsimd): packed (j, s)-tile-major layout,
            # one contiguous 256KB dma_start per (pass, j)
            xkv_chunks = {}

            def prefetch_xkv(ms, js):
                for j in js:
                    xkv_ch = xkvp.tile([128, len(ms), 128], bf16, tag="xkv",
                                       name=f"xkv{ms[0]}_{j}")
                    nc.gpsimd.dma_start(
                        xkv_ch[:],
                        xkv_d.ap()[:, j * NT_S * 128 + ms[0] * 128:
                                   j * NT_S * 128 + (ms[-1] + 1) * 128])
                    xkv_chunks[(j, ms[0])] = xkv_ch

            # first-half xkv for stage 3a: issue the first 4 (= pool depth)
            # before the collective occupies the gpsimd stream
            local_ms = list(range(NT_L)) if dist else list(range(NT_S))
            prefetch_xkv(local_ms, range(4))

            # ---- stage 1b: KT (local half in dist mode) -> kt_sb ----
            # split-outer/chunk-outer: the first d_key row-pairs finish first
            # (feeding AllGather #1 early) and the first groups need only
            # chunk 0; cc_in row-stores go out inline per row block.
            for half in range(NSPLIT):
                ilist = range(half * IPG, (half + 1) * IPG)
                for c in range(len(KCH)):
                    w, off = KCH[c], KOFF[c]
                    for i in ilist:
                        ps = psp.tile([128, 512], fp32, tag="ps")
                        for j in range(NT_D):
                            nc.tensor.matmul(
                                ps[:, :w],
                                wk_blks[i][:, j * 128:(j + 1) * 128],
                                xk_chs[c][:, j, :],
                                start=(j == 0), stop=(j == NT_D - 1))
                        if with_bk:
                            nc.scalar.activation(
                                kt_sb[:, i, off:off + w], ps[:, :w],
                                mybir.ActivationFunctionType.Identity,
                                bias=bk_sb[:, i:i + 1])
                        else:
                            nc.scalar.activation(
                                kt_sb[:, i, off:off + w], ps[:, :w],
                                mybir.ActivationFunctionType.Copy)
                        if dist and c == LAST_C:
                            (nc.sync if i % 2 == 0 else nc.scalar).dma_start(
                                cc_ins[half].ap()[(i % IPG) * 128:
                                                  (i % IPG + 1) * 128, :],
                                kt_sb[:, i, 0:QL])
                if dist:
                    nc.gpsimd.collective_compute(
                        "AllGather", mybir.AluOpType.bypass,
                        replica_groups=groups,
                        ins=[cc_ins[half].ap()[:].opt()],
                        outs=[cc_outs[half].ap()[:].opt()],
                    )
                if half == 0:
                    # deferred xq chunk 1 (needed ~30us later): issues once
                    # the startup-critical bytes have cleared the wire
                    ld_xq(sy, 1, 0)
                    ld_xq(gp, 1, 1)

            # ---- stage 1a: QT -> qt_sb ----
            qt_sb = bigp.tile([128, NT_D, D], bf16, tag="big", name="qt_sb")
            for n in range(NQ):
                for i in range(NT_D):
                    ps = psp.tile([128, 512], fp32, tag="ps")
                    for j in range(NT_D):
                        nc.tensor.matmul(
                            ps[:], wq_blks[i][:, j * 128:(j + 1) * 128],
                            xq_chs[n][:, j, :],
                            start=(j == 0), stop=(j == NT_D - 1))
                    if with_bq:
                        nc.scalar.activation(
                            qt_sb[:, i, n * 512:(n + 1) * 512], ps[:],
                            mybir.ActivationFunctionType.Identity,
                            bias=bq_sb[:, i:i + 1])
                    else:
                        nc.scalar.activation(
                            qt_sb[:, i, n * 512:(n + 1) * 512], ps[:],
                            mybir.ActivationFunctionType.Copy)

            if dist:
                # read the gathered pair back, recover the partner's block via
                # remote = g0 ^ g1 ^ local (exact bf16 bit identity) -> ktr_sb.
                # sync + gpsimd (both are past their other issue duties; the
                # collective-wait embedded in these DMAs blocks the engine,
                # so they must never sit on the scalar stream). s-half-outer:
                # the first remote score groups need only ktr's leading 512
                # s-columns, so they unblock after half the readback bytes.
                for h2 in range(2):
                    lo = h2 * 512
                    for i in range(NT_D):
                        cc_view = cc_outs[i // IPG].ap().rearrange(
                            "(b r) f -> r b f", b=2)
                        r0 = (i % IPG) * 128
                        g_ch = gchp.tile([128, 2, 512], bf16, tag="gch",
                                         name=f"gch{i}_{h2}")
                        eng = nc.sync if i % 2 == 0 else nc.gpsimd
                        eng.dma_start(
                            g_ch[:, 0, :],
                            cc_view[r0:r0 + 128, 0, lo:lo + 512])
                        eng.dma_start(
                            g_ch[:, 1, :],
                            cc_view[r0:r0 + 128, 1, lo:lo + 512])
                        nc.vector.tensor_tensor(
                            g_ch[:, 0, :].bitcast(uint32),
                            g_ch[:, 0, :].bitcast(uint32),
                            g_ch[:, 1, :].bitcast(uint32),
                            mybir.AluOpType.bitwise_xor)
                        nc.vector.tensor_tensor(
                            ktr_sb[:, i, lo:lo + 512].bitcast(uint32),
                            g_ch[:, 0, :].bitcast(uint32),
                            kt_sb[:, i, lo:lo + 512].bitcast(uint32),
                            mybir.AluOpType.bitwise_xor)

            # ---- stage 2: scores^T + exp ----
            def score_group(m, n):
                kt, mm = (ktr_sb, m - NT_L) if (dist and m >= NT_L) else (kt_sb, m)
                ps = psp.tile([128, 512], fp32, tag="ps")
                for i in range(NT_D):
                    nc.tensor.matmul(
                        ps[:], kt[:, i, mm * 128:(mm + 1) * 128],
                        qt_sb[:, i, n * 512:(n + 1) * 512],
                        start=(i == 0), stop=(i == NT_D - 1))
                if with_mask:
                    mk = maskp.tile([128, 512], bf16, tag="mask")
                    nc.sync.dma_start(
                        mk[:], maskt_d.ap()[m * 128:(m + 1) * 128,
                                            n * 512:(n + 1) * 512])
                    nc.vector.tensor_tensor(
                        ps[:], ps[:], mk[:], mybir.AluOpType.add)
                nc.scalar.activation(
                    et_sb[:, m, n * 512:(n + 1) * 512], ps[:],
                    mybir.ActivationFunctionType.Exp, scale=SCALE)

            first_ms = range(NT_L) if dist else range(NT_S)
            for n in range(NQ):
                for m in first_ms:
                    score_group(m, n)

            # softmax denominators: accumulate expT tiles on the DVE (PE
            # has no slack; DVE has plenty). In-place fp32 chain.
            sacc = cons.tile([128, QL], fp32, tag="sacc")
            first_l = list(first_ms)
            nc.vector.tensor_tensor(
                sacc[:], et_sb[:, first_l[0], :], et_sb[:, first_l[1], :],
                mybir.AluOpType.add)
            for m in first_l[2:]:
                nc.vector.tensor_tensor(
                    sacc[:], sacc[:], et_sb[:, m, :], mybir.AluOpType.add)

            # ---- stage 3a: HT over available s-tiles ----
            ht_sb = bigp.tile([128, NT_D, D], bf16, tag="big", name="ht_sb")

            def ht_groups(ms, merge, skip_prefetch=0):
                prefetch_xkv(ms, range(skip_prefetch, NT_D))
                for j in range(NT_D):
                    xkv_ch = xkv_chunks[(j, ms[0])]
                    for n in range(NQ):
                        ps = psp.tile([128, 512], fp32, tag="ps")
                        for k2, m in enumerate(ms):
                            nc.tensor.matmul(
                                ps[:], xkv_ch[:, k2, :],
                                et_sb[:, m, n * 512:(n + 1) * 512],
                                start=(k2 == 0), stop=(k2 == len(ms) - 1))
                        dst = ht_sb[:, j, n * 512:(n + 1) * 512]
                        if merge:
                            nc.vector.tensor_tensor(
                                dst, ps[:], dst, mybir.AluOpType.add)
                        else:
                            nc.scalar.activation(
                                dst, ps[:],
                                mybir.ActivationFunctionType.Copy)

            if dist:
                ht_groups(local_ms, merge=False, skip_prefetch=4)
                # m-outer: exp(m) completes both q-chunks back-to-back so the
                # DVE sums chain below never lags the PE
                for m in range(NT_L, NT_S):
                    for n in range(NQ):
                        score_group(m, n)
                for m in range(NT_L, NT_S):
                    nc.vector.tensor_tensor(
                        sacc[:], sacc[:], et_sb[:, m, :], mybir.AluOpType.add)
                ht_groups(list(range(NT_L, NT_S)), merge=True)
            else:
                ht_groups(local_ms, merge=False, skip_prefetch=4)

            # sums[1, q]: single fp32 ones-matmul per q-chunk over sacc.
            # Allocated here; EMITTED inside stage 4 after the first output
            # group so the PE chews useful matmuls while the DVE chain ends.
            sums_sb = cons.tile([1, QL], fp32, tag="sums")
            pst = pstp.tile([128, 8], fp32, tag="pst")
            recip_sb = cons.tile([128, 8], fp32, tag="recip")
            if with_bv:
                sums_bf = cons.tile([1, QL], bf16, tag="sums_bf")

            def emit_sums():
                for n in range(NQ):
                    pss = pssp.tile([1, 512], fp32, tag="pss")
                    nc.tensor.matmul(
                        pss[:], ones_col[:], sacc[:, n * 512:(n + 1) * 512],
                        start=True, stop=True)
                    nc.scalar.activation(
                        sums_sb[:, n * 512:(n + 1) * 512], pss[:],
                        mybir.ActivationFunctionType.Copy)
                for p in range(8):
                    nc.tensor.transpose(
                        pst[:, p:p + 1], sums_sb[:, p * 128:(p + 1) * 128],
                        ident1[:])
                nc.vector.reciprocal(recip_sb[:], pst[:])
                if with_bv:
                    # out accumulates UNNORMALIZED; bias enters as sums[q]*bv
                    # so the final 1/sums scale leaves exactly +bv
                    nc.scalar.activation(sums_bf[:], sums_sb[:],
                                         mybir.ActivationFunctionType.Copy)

            # ---- stage 4: out = HT^T . Wv (+bv), normalized, bf16 out ----
            # p=0: matmuls first, then the sums block (PE stays busy while
            # the DVE chain finishes), then the p=0 normalization. The last
            # p runs its final 512 cols as two 256-col groups with act+store
            # flushed per group, so only one short act+DMA chain trails the
            # last matmul.
            for p in range(8):
                out_sb = outp.tile([128, D], bf16, tag="outsb")
                widths = ([(0, 512), (512, 512)] if p < 7
                          else [(0, 512), (512, 256), (768, 256)])
                pending = []

                def flush(p=p, out_sb=out_sb, pending=pending):
                    for k, (ps, lo, w) in enumerate(pending):
                        if with_bv:
                            nc.tensor.matmul(
                                ps[:, :w], sums_bf[:, p * 128:(p + 1) * 128],
                                bv_sb[:, lo:lo + w],
                                start=False, stop=True)
                        nc.scalar.activation(
                            out_sb[:, lo:lo + w], ps[:, :w],
                            mybir.ActivationFunctionType.Copy,
                            scale=recip_sb[:, p:p + 1])
                        # p=7 stores all go on sync: a store issued on the
                        # scalar stream would delay the final activations
                        # (~0.7us issue op each) right at the kernel tail
                        eng = (nc.sync if p == 7 or (p + lo // 512) % 2 == 0
                               else nc.scalar)
                        eng.dma_start(
                            out_d.ap()[p * 128:(p + 1) * 128, lo:lo + w],
                            out_sb[:, lo:lo + w])
                    pending.clear()

                for lo, w in widths:
                    ps = psp.tile([128, 512], fp32, tag="ps")
                    for j in range(NT_D):
                        nc.tensor.matmul(
                            ps[:, :w], ht_sb[:, j, p * 128:(p + 1) * 128],
                            wv_sb[:, j, lo:lo + w],
                            start=(j == 0),
                            stop=(j == NT_D - 1 and not with_bv))
                    pending.append((ps, lo, w))
                    if p > 0:
                        flush()
                if p == 0:
                    emit_sums()
                    flush()

    nc.compile()
    return nc


def _get_nc(flags):
    if flags not in _cache:
        _cache[flags] = _build(*flags)
    return _cache[flags]


def _flags_of(inputs, dist=True):
    return _prep_in_maps(**inputs, dist=dist)[0]


def _pack_w(W):
    # [j*128+p, i*128+c] -> [p, i*1024 + j*128 + c]
    return np.ascontiguousarray(
        W.reshape(8, 128, 8, 128).transpose(1, 2, 0, 3).reshape(128, 8192))


def _pack_x(xt, widths):
    # xt [D, L] (row-major) -> [128, sum_c 8*w_c]: for each col-chunk c,
    # block[p, j*w + t] = xt[j*128+p, off+t]  (SBUF tile layout, so each
    # chunk is one fully-contiguous DMA)
    blocks = []
    off = 0
    for w in widths:
        blk = xt[:, off:off + w].reshape(8, 128, w).transpose(1, 0, 2)
        blocks.append(blk.reshape(128, 8 * w))
        off += w
    return np.ascontiguousarray(np.concatenate(blocks, axis=1))


def _pack_kv(xkv):
    # xkv [S, D] -> [128, j*S + m*128 + t] with xkv[m*128+p, j*128+t]:
    # per (s-pass, j) loads are fully contiguous
    nts = xkv.shape[0] // 128
    return np.ascontiguousarray(
        xkv.reshape(nts, 128, 8, 128).transpose(1, 2, 0, 3)
        .reshape(128, nts * 1024))


def _prep_in_maps(query_input, keyvalue_input, mask, Wq, bq, Wk, bk, Wv, bv,
                  dist=True):
    qi = np.asarray(query_input, np.float32)
    kv = np.asarray(keyvalue_input, np.float32)
    mask = np.asarray(mask, np.float32)
    Wqb = np.asarray(Wq, np.float32).astype(BF16)
    Wkb = np.asarray(Wk, np.float32).astype(BF16)
    Wvb = np.asarray(Wv, np.float32).astype(BF16)
    bq = np.asarray(bq, np.float32)
    bk = np.asarray(bk, np.float32)
    bv = np.asarray(bv, np.float32)

    with_mask = bool(np.any(mask != 0.0))
    with_bq = bool(np.any(bq != 0.0))
    with_bk = bool(np.any(bk != 0.0))
    with_bv = bool(np.any(bv != 0.0))
    flags = (dist, with_mask, with_bq, with_bk, with_bv)

    Wq_p = _pack_w(Wqb)
    Wk_p = _pack_w(Wkb)

    in_maps = []
    for c in range(N_CORES):
        b, h = c // 2, c % 2
        xq = qi[b, h * QL:(h + 1) * QL, :].astype(BF16)       # [QL, D]
        xkv = kv[b].astype(BF16)                               # [S, D]
        if dist:
            xkvt = np.ascontiguousarray(xkv[h * QL:(h + 1) * QL, :].T)
            perm_kv = np.concatenate(
                [xkv[h * QL:(h + 1) * QL], xkv[(1 - h) * QL:(2 - h) * QL]])
        else:
            xkvt = np.ascontiguousarray(xkv.T)
            perm_kv = xkv
        m = {
            "xqt": _pack_x(np.ascontiguousarray(xq.T), [512, 512]),
            "xkvt": _pack_x(xkvt, _kch(xkvt.shape[1])),
            "xkv": _pack_kv(np.ascontiguousarray(perm_kv)),
            "wq": Wq_p, "wk": Wk_p, "wv": Wvb,
        }
        if with_bq:
            m["bq"] = np.ascontiguousarray(bq.reshape(8, 128).T)
        if with_bk:
            m["bk"] = np.ascontiguousarray(bk.reshape(8, 128).T)
        if with_bv:
            m["bv"] = bv.astype(BF16).reshape(1, D)
        if with_mask:
            mt = mask[b, h * QL:(h + 1) * QL, :].T * np.float32(np.sqrt(D))
            if dist:
                mt = np.concatenate(
                    [mt[h * QL:(h + 1) * QL], mt[(1 - h) * QL:(2 - h) * QL]])
            m["maskt"] = np.ascontiguousarray(mt.astype(np.float32)).astype(BF16)
        in_maps.append(m)
    return flags, in_maps


def _ensure_axon_hooks_stub():
    # bass_utils imports antenv.axon_hooks when tracing is requested (even via
    # a stray BASS_TRACE env var); the module is absent on some images, so
    # register a no-op stub if needed.
    import sys, types
    try:
        import antenv.axon_hooks  # noqa: F401
    except ImportError:
        stub = types.ModuleType("antenv.axon_hooks")
        stub._hook = None
        stub.set_axon_ntff_profile_hook = (
            lambda h: setattr(stub, "_hook", h))
        stub.get_axon_ntff_profile_hook = lambda: stub._hook
        sys.modules["antenv.axon_hooks"] = stub
        try:
            import antenv
            antenv.axon_hooks = stub
        except ImportError:
            pass


def _run(inputs, trace=False, **kw):
    _ensure_axon_hooks_stub()
    from concourse import bass_utils
    dist = os.environ.get("KERNEL_DIST", "1") == "1"
    ps_bufs = int(os.environ.get("KERNEL_PSBUFS", "5"))
    flags, in_maps = _prep_in_maps(**inputs, dist=dist)
    nc = _get_nc(flags + (ps_bufs,))
    res = bass_utils.run_bass_kernel_spmd(
        nc, in_maps, core_ids=list(range(N_CORES)), trace=trace, **kw)
    out = np.empty((B, S, D), np.float32)
    for c in range(N_CORES):
        b, h = c // 2, c % 2
        out[b, h * QL:(h + 1) * QL, :] = np.asarray(
            res.results[c]["out"], dtype=np.float32)
    return out, res


def kernel(**inputs) -> np.ndarray:
    out, _ = _run(inputs, trace=False)
    return out



# revision 3
# speedup vs baseline: 1.1282x; 1.1282x over previous
"""Trainium2 Bass kernel: single-head attention (B=4, S=2048, D=1024) on 8 NeuronCores.

Sharding: data-parallel over (batch, query-half): core c handles batch c//2,
query rows [c%2*1024, (c%2+1)*1024). No collectives.

Key algebraic fold: with zero q/k biases,
  scores = (Xq Wq)(Xkv Wk)^T = Xq (Wq Wk^T) Xkv^T = Xq M Xkv^T
with M = Wq Wk^T precomputed once on host (weights-only). This removes the
K-projection (2.1 GF/core) and the K^T AllGather of the previous design.
Per-core FLOPs: 12.88 GF -> 164 us PE floor at 78.6 TF/s bf16.

Nonzero bq/bk are handled by the augmented form (rare path, na=9 tiles):
  scores = [Xq 1] [[M, Wq bk],[(Wk bq)^T, bq.bk]] [Xkv 1]^T
zero-padded from 1025 to 1152 rows/cols. Mask and bv keep their own paths
(mask: pre-scaled add before exp; bv: out accumulates unnormalized and bias
enters as sums[q]*bv so the final 1/sums scale leaves exactly +bv).

Math per core (all matmuls bf16, fp32 PSUM accumulation):
  PT[k,q]   = M-blocks(lhsT) . XqT(rhs)           P = Xq M
  sT[s,q]   = XkvT-tiles(lhsT) . PT(rhs)          scores^T
  eT[s,q]   = exp(sT / sqrt(dk) [+ maskT])        ScalarE, PSUM->SBUF bf16
  sums      = DVE adds over eT s-tiles, then ones-matmul + reciprocal
  HT[v,q]   = Xkv-tiles(lhsT) . eT(rhs)           H = probs_unnorm @ Xkv
  out[q,dv] = HT-tiles(lhsT) . Wv(rhs)  (+bv)     out = H @ Wv
  out      *= 1/sums  (per-partition scale on ScalarE, fused with PSUM->SBUF)

PE group order (zero inter-stage stalls by construction: each stage's first
group depends only on work finished many groups earlier):
  A(c=0..2, i)  ->  B(n=0, m=0..15), B(n=1, m)  ->  D(n=0, j), D(n=1, j)
  ->  E(p=0..3), E(p=4..7); sums emitted inside E p=0.
Stage A runs chunk widths [256,256,512]: the first groups need only
m-block 0 + a 512KB lead of XqT, shrinking the startup-critical DMA set.

DMA design (from v1 trace analysis): aggregate startup bandwidth scales with
in-flight dma_starts; keep payloads ~128-512KB, fan out across sync, gpsimd
and vector queues (scalar gets only 2 issues so its first activation isn't
delayed; vector is free until the sacc chain at ~85us). All inputs are
host-packed into SBUF tile layouts so every dma_start is fully contiguous.
"""

import os
import numpy as np
import ml_dtypes

B, S, D = 4, 2048, 1024
N_CORES = 8
QL = S // 2  # query rows per core (1024)
NT_S = S // 128  # 16 s-tiles
BF16 = ml_dtypes.bfloat16

_cache: dict = {}

AQCH = [256, 256, 512]  # stage-A chunk widths (sum = QL... per 1024 q cols)
AOFF = [0, 256, 512]


def _build(na: int, with_mask: bool, with_bv: bool, ps_bufs: int = 5):
    import concourse.bass as bass
    import concourse.mybir as mybir
    import concourse.tile as tile
    from concourse import bacc

    fp32 = mybir.dt.float32
    bf16 = mybir.dt.bfloat16

    nc = bacc.Bacc("TRN2", target_bir_lowering=False, debug=False,
                   num_devices=N_CORES)

    KA = na * 128          # augmented d_model/d_key (1024 or 1152)
    NQ = QL // 512         # 2 query chunks of 512
    NV = D // 128          # 8 v-blocks
    SCALE = 1.0 / float(np.sqrt(D))

    m_d = nc.dram_tensor("m", (128, na * KA), bf16, kind="ExternalInput")
    xq_d = nc.dram_tensor("xqt", (128, na * QL), bf16, kind="ExternalInput")
    kt_d = nc.dram_tensor("xkvt", (128, NT_S * KA), bf16, kind="ExternalInput")
    xkv_d = nc.dram_tensor("xkv", (128, NV * S), bf16, kind="ExternalInput")
    wv_d = nc.dram_tensor("wv", (128, NV * D), bf16, kind="ExternalInput")
    if with_bv:
        bv_d = nc.dram_tensor("bv", (1, D), bf16, kind="ExternalInput")
    if with_mask:
        maskt_d = nc.dram_tensor("maskt", (S, QL), bf16, kind="ExternalInput")
    out_d = nc.dram_tensor("out", (QL, D), bf16, kind="ExternalOutput")

    with tile.TileContext(nc) as tc:
        with (
            tc.tile_pool(name="cons", bufs=1) as cons,
            tc.tile_pool(name="mp", bufs=1) as mp,
            tc.tile_pool(name="xqp", bufs=1) as xqp,
            tc.tile_pool(name="ktp", bufs=1) as ktp,
            tc.tile_pool(name="ptp", bufs=1) as ptp,
            tc.tile_pool(name="etp", bufs=1) as etp,
            tc.tile_pool(name="xkvp", bufs=1) as xkvp,
            tc.tile_pool(name="wvp", bufs=1) as wvp,
            tc.tile_pool(name="htp", bufs=1) as htp,
            tc.tile_pool(name="outp", bufs=2) as outp,
            tc.tile_pool(name="maskp", bufs=2) as maskp,
            tc.tile_pool(name="ps", bufs=ps_bufs,
                         space=bass.MemorySpace.PSUM) as psp,
            tc.tile_pool(name="pss", bufs=2, space=bass.MemorySpace.PSUM) as pssp,
            tc.tile_pool(name="pst", bufs=1, space=bass.MemorySpace.PSUM) as pstp,
        ):
            sy, sc, gp, ve = nc.sync, nc.scalar, nc.gpsimd, nc.vector

            # ---- SBUF tiles ----
            m_blks = [mp.tile([128, KA], bf16, tag=f"m{i}", name=f"m{i}")
                      for i in range(na)]
            xq_chs = [xqp.tile([128, na, AQCH[c]], bf16, tag=f"xq{c}",
                               name=f"xq{c}") for c in range(3)]
            kt_sb = ktp.tile([128, NT_S, KA], bf16, tag="kt")
            pt_sb = ptp.tile([128, na, QL], bf16, tag="pt")
            et_sb = etp.tile([128, NT_S, QL], bf16, tag="et")
            xkv_sb = xkvp.tile([128, NV, NT_S, 128], bf16, tag="xkv")
            wv_sb = wvp.tile([128, NV, D], bf16, tag="wv")
            ht_sb = htp.tile([128, NV, QL], bf16, tag="ht")

            # ---- DMA issue schedule (priority order per engine) ----
            def ld_m(eng, i, lo=0, hi=None):
                hi = KA if hi is None else hi
                eng.dma_start(m_blks[i][:, lo:hi],
                              m_d.ap()[:, i * KA + lo:i * KA + hi])

            def ld_xq(eng, c, jlo, jhi):
                w = AQCH[c]
                base = na * AOFF[c]
                eng.dma_start(xq_chs[c][:, jlo:jhi, :],
                              xq_d.ap()[:, base + jlo * w:base + jhi * w])

            def ld_kt(eng, mlo, mhi):
                eng.dma_start(kt_sb[:, mlo:mhi, :],
                              kt_d.ap()[:, mlo * KA:mhi * KA])

            def ld_xkv(eng, j):
                eng.dma_start(xkv_sb[:, j, :, :],
                              xkv_d.ap()[:, j * S:(j + 1) * S])

            def ld_wv(eng, jlo, jhi):
                eng.dma_start(wv_sb[:, jlo:jhi, :],
                              wv_d.ap()[:, jlo * D:jhi * D])

            # startup-critical wave: first A group needs m0 + xq chunk 0.
            # tiny dedicated leads put the first matmul ~2us earlier (cold
            # rings run ~100GB/s), the rest follows right behind.
            sy.dma_start(m_blks[0][:, 0:128], m_d.ap()[:, 0:128])
            gp.dma_start(xq_chs[0][:, 0:1, :], xq_d.ap()[:, 0:AQCH[0]])
            sc.dma_start(m_blks[0][:, 128:512], m_d.ap()[:, 128:512])
            sy.dma_start(m_blks[0][:, 512:KA], m_d.ap()[:, 512:KA])
            gp.dma_start(xq_chs[0][:, 1:4, :],
                         xq_d.ap()[:, AQCH[0]:4 * AQCH[0]])
            sc.dma_start(xq_chs[0][:, 4:na, :],
                         xq_d.ap()[:, 4 * AQCH[0]:na * AQCH[0]])
            ld_m(sy, 1)
            ld_m(gp, 2)
            # chunk 1 feeds A groups ~9us in; chunk 2 ~18us in
            ld_xq(sy, 1, 0, 4); ld_xq(gp, 1, 4, na)
            ld_m(sy, 3); ld_m(gp, 4)
            ld_xq(sy, 2, 0, 4); ld_xq(gp, 2, 4, na)
            ld_m(sy, 5); ld_m(gp, 6); ld_m(sy, 7)
            if na > 8:
                ld_m(gp, 8)

            # stage-B operands: 16 s-blocks consumed from ~28us, sequential
            for m4 in range(0, NT_S, 4):
                ld_kt(sy, m4, m4 + 2)
                ld_kt(gp, m4 + 2, m4 + 4)

            # stage-D operands (from ~82us) and stage-E weights (~137us)
            for j in range(NV):
                ld_xkv((sy, gp)[j % 2], j)
            ld_wv(gp, 0, 2); ld_wv(sy, 2, 4)
            ld_wv(gp, 4, 6); ld_wv(sy, 6, 8)

            # ---- constants (tiny, off the critical path) ----
            ones_col = cons.tile([128, 1], fp32, tag="ones_col")
            gp.memset(ones_col[:], 1.0)
            ident1 = cons.tile([1, 1], fp32, tag="ident1")
            gp.memset(ident1[:], 1.0)
            if with_bv:
                bv_sb = cons.tile([1, D], bf16, tag="bv")
                sy.dma_start(bv_sb[:], bv_d.ap()[:])

            # ---- stage A: PT = M-blocks . XqT ----
            for c in range(3):
                w, off = AQCH[c], AOFF[c]
                for i in range(na):
                    ps = psp.tile([128, 512], fp32, tag="ps")
                    for j in range(na):
                        nc.tensor.matmul(
                            ps[:, :w], m_blks[i][:, j * 128:(j + 1) * 128],
                            xq_chs[c][:, j, :],
                            start=(j == 0), stop=(j == na - 1))
                    nc.scalar.activation(
                        pt_sb[:, i, off:off + w], ps[:, :w],
                        mybir.ActivationFunctionType.Copy)

            # ---- stage B: scores^T + exp ----
            for n in range(NQ):
                for m in range(NT_S):
                    ps = psp.tile([128, 512], fp32, tag="ps")
                    for j in range(na):
                        nc.tensor.matmul(
                            ps[:], kt_sb[:, m, j * 128:(j + 1) * 128],
                            pt_sb[:, j, n * 512:(n + 1) * 512],
                            start=(j == 0), stop=(j == na - 1))
                    if with_mask:
                        mk = maskp.tile([128, 512], bf16, tag="mask")
                        sy.dma_start(
                            mk[:], maskt_d.ap()[m * 128:(m + 1) * 128,
                                                n * 512:(n + 1) * 512])
                        nc.vector.tensor_tensor(
                            ps[:], ps[:], mk[:], mybir.AluOpType.add)
                    nc.scalar.activation(
                        et_sb[:, m, n * 512:(n + 1) * 512], ps[:],
                        mybir.ActivationFunctionType.Exp, scale=SCALE)

            # softmax denominators on the DVE (PE has no slack; DVE does).
            sacc = cons.tile([128, QL], fp32, tag="sacc")
            nc.vector.tensor_tensor(
                sacc[:], et_sb[:, 0, :], et_sb[:, 1, :], mybir.AluOpType.add)
            for m in range(2, NT_S):
                nc.vector.tensor_tensor(
                    sacc[:], sacc[:], et_sb[:, m, :], mybir.AluOpType.add)

            # ---- stage D: HT = Xkv-tiles . eT ----
            for n in range(NQ):
                for j in range(NV):
                    ps = psp.tile([128, 512], fp32, tag="ps")
                    for m in range(NT_S):
                        nc.tensor.matmul(
                            ps[:], xkv_sb[:, j, m, :],
                            et_sb[:, m, n * 512:(n + 1) * 512],
                            start=(m == 0), stop=(m == NT_S - 1))
                    nc.scalar.activation(
                        ht_sb[:, j, n * 512:(n + 1) * 512], ps[:],
                        mybir.ActivationFunctionType.Copy)

            # sums block: emitted inside stage E after p=0's matmuls so the
            # PE chews useful work while the DVE chain ends.
            sums_sb = cons.tile([1, QL], fp32, tag="sums")
            pst = pstp.tile([128, 8], fp32, tag="pst")
            recip_sb = cons.tile([128, 8], fp32, tag="recip")
            if with_bv:
                sums_bf = cons.tile([1, QL], bf16, tag="sums_bf")

            def emit_sums():
                for n in range(NQ):
                    pss = pssp.tile([1, 512], fp32, tag="pss")
                    nc.tensor.matmul(
                        pss[:], ones_col[:], sacc[:, n * 512:(n + 1) * 512],
                        start=True, stop=True)
                    nc.scalar.activation(
                        sums_sb[:, n * 512:(n + 1) * 512], pss[:],
                        mybir.ActivationFunctionType.Copy)
                for p in range(8):
                    nc.tensor.transpose(
                        pst[:, p:p + 1], sums_sb[:, p * 128:(p + 1) * 128],
                        ident1[:])
                nc.vector.reciprocal(recip_sb[:], pst[:])
                if with_bv:
                    # out accumulates UNNORMALIZED; bias enters as sums[q]*bv
                    # so the final 1/sums scale leaves exactly +bv
                    nc.scalar.activation(sums_bf[:], sums_sb[:],
                                         mybir.ActivationFunctionType.Copy)

            # ---- stage E: out = HT^T . Wv (+bv), normalized, bf16 out ----
            # p=0: matmuls first, then the sums block, then normalization.
            # The last p runs its final 512 cols as two 256-col groups with
            # act+store flushed per group, so only one short act+DMA chain
            # trails the last matmul.
            for p in range(8):
                out_sb = outp.tile([128, D], bf16, tag="outsb")
                widths = ([(0, 512), (512, 512)] if p < 7
                          else [(0, 512), (512, 256), (768, 256)])
                pending = []

                def flush(p=p, out_sb=out_sb, pending=pending):
                    for ps, lo, w in pending:
                        if with_bv:
                            nc.tensor.matmul(
                                ps[:, :w], sums_bf[:, p * 128:(p + 1) * 128],
                                bv_sb[:, lo:lo + w],
                                start=False, stop=True)
                        nc.scalar.activation(
                            out_sb[:, lo:lo + w], ps[:, :w],
                            mybir.ActivationFunctionType.Copy,
                            scale=recip_sb[:, p:p + 1])
                        # p=7 stores all go on sync: a store issued on the
                        # scalar stream would delay the final activations
                        eng = (nc.sync if p == 7 or (p + lo // 512) % 2 == 0
                               else nc.scalar)
                        eng.dma_start(
                            out_d.ap()[p * 128:(p + 1) * 128, lo:lo + w],
                            out_sb[:, lo:lo + w])
                    pending.clear()

                for lo, w in widths:
                    ps = psp.tile([128, 512], fp32, tag="ps")
                    for j in range(NV):
                        nc.tensor.matmul(
                            ps[:, :w], ht_sb[:, j, p * 128:(p + 1) * 128],
                            wv_sb[:, j, lo:lo + w],
                            start=(j == 0),
                            stop=(j == NV - 1 and not with_bv))
                    pending.append((ps, lo, w))
                    if p > 0:
                        flush()
                if p == 0:
                    emit_sums()
                    flush()

    nc.compile()
    return nc


def _get_nc(flags):
    if flags not in _cache:
        _cache[flags] = _build(*flags)
    return _cache[flags]


def _pack_blocks(X, nj, ni):
    # X [nj*128, ni*128] -> [128, ni*nj*128]: out[p, (i*nj+j)*128+c]
    # = X[j*128+p, i*128+c] (i-block-major SBUF tile layout, each i-block
    # one fully-contiguous DMA)
    return np.ascontiguousarray(
        X.reshape(nj, 128, ni, 128).transpose(1, 2, 0, 3)
        .reshape(128, ni * nj * 128))


def _pack_x(xt, widths, na):
    # xt [na*128, L] -> [128, sum_c na*w_c]: per col-chunk c,
    # block[p, j*w + t] = xt[j*128+p, off+t]
    blocks = []
    off = 0
    for w in widths:
        blk = xt[:, off:off + w].reshape(na, 128, w).transpose(1, 0, 2)
        blocks.append(blk.reshape(128, na * w))
        off += w
    return np.ascontiguousarray(np.concatenate(blocks, axis=1))


def _prep_in_maps(query_input, keyvalue_input, mask, Wq, bq, Wk, bk, Wv, bv):
    qi = np.asarray(query_input, np.float32)
    kv = np.asarray(keyvalue_input, np.float32)
    mask = np.asarray(mask, np.float32)
    Wq = np.asarray(Wq, np.float32)
    Wk = np.asarray(Wk, np.float32)
    Wvb = np.asarray(Wv, np.float32).astype(BF16)
    bq = np.asarray(bq, np.float32)
    bk = np.asarray(bk, np.float32)
    bv = np.asarray(bv, np.float32)

    with_mask = bool(np.any(mask != 0.0))
    with_bias = bool(np.any(bq != 0.0) or np.any(bk != 0.0))
    with_bv = bool(np.any(bv != 0.0))
    na = 9 if with_bias else 8
    KA = na * 128
    flags = (na, with_mask, with_bv)

    # M = Wq Wk^T (weights-only fold), augmented with q/k biases if nonzero
    M = np.zeros((KA, KA), np.float32)
    M[:D, :D] = Wq @ Wk.T
    if with_bias:
        M[:D, D] = Wq @ bk
        M[D, :D] = Wk @ bq
        M[D, D] = float(bq @ bk)
    M_p = _pack_blocks(M.astype(BF16), na, na)
    wv_p = np.ascontiguousarray(
        Wvb.reshape(8, 128, D).transpose(1, 0, 2).reshape(128, 8 * D))

    in_maps = []
    for c in range(N_CORES):
        b, h = c // 2, c % 2
        xq = qi[b, h * QL:(h + 1) * QL, :]          # [QL, D] fp32
        xkv = kv[b]                                  # [S, D] fp32
        xqt = np.zeros((KA, QL), np.float32)
        xqt[:D] = xq.T
        xkvt = np.zeros((KA, S), np.float32)
        xkvt[:D] = xkv.T
        if with_bias:
            xqt[D] = 1.0
            xkvt[D] = 1.0
        m_ = {
            "m": M_p,
            "xqt": _pack_x(xqt.astype(BF16), AQCH, na),
            "xkvt": _pack_blocks(xkvt.astype(BF16), na, NT_S),
            "xkv": _pack_blocks(xkv.astype(BF16), NT_S, 8),
            "wv": wv_p,
        }
        if with_bv:
            m_["bv"] = bv.astype(BF16).reshape(1, D)
        if with_mask:
            mt = mask[b, h * QL:(h + 1) * QL, :].T * np.float32(np.sqrt(D))
            m_["maskt"] = np.ascontiguousarray(mt.astype(BF16))
        in_maps.append(m_)
    return flags, in_maps


def _ensure_axon_hooks_stub():
    # bass_utils imports antenv.axon_hooks when tracing is requested; the
    # module is absent on some images, so register a no-op stub if needed.
    import sys, types
    try:
        import antenv.axon_hooks  # noqa: F401
    except ImportError:
        stub = types.ModuleType("antenv.axon_hooks")
        stub._hook = None
        stub.set_axon_ntff_profile_hook = (
            lambda h: setattr(stub, "_hook", h))
        stub.get_axon_ntff_profile_hook = lambda: stub._hook
        sys.modules["antenv.axon_hooks"] = stub
        try:
            import antenv
            antenv.axon_hooks = stub
        except ImportError:
            pass


def _run(inputs, trace=False, **kw):
    _ensure_axon_hooks_stub()
    from concourse import bass_utils
    ps_bufs = int(os.environ.get("KERNEL_PSBUFS", "5"))
    flags, in_maps = _prep_in_maps(**inputs)
    nc = _get_nc(flags + (ps_bufs,))
    res = bass_utils.run_bass_kernel_spmd(
        nc, in_maps, core_ids=list(range(N_CORES)), trace=trace, **kw)
    out = np.empty((B, S, D), np.float32)
    for c in range(N_CORES):
        b, h = c // 2, c % 2
        out[b, h * QL:(h + 1) * QL, :] = np.asarray(
            res.results[c]["out"], dtype=np.float32)
    return out, res


def kernel(**inputs) -> np.ndarray:
    out, _ = _run(inputs, trace=False)
    return out


# revision 10
# speedup vs baseline: 1.1463x; 1.0161x over previous
"""Trainium2 Bass kernel: single-head attention (B=4, S=2048, D=1024) on 8 NeuronCores.

Sharding: data-parallel over (batch, query-half): core c handles batch c//2,
query rows [c%2*1024, (c%2+1)*1024). No collectives.

Key algebraic fold: with zero q/k biases,
  scores = (Xq Wq)(Xkv Wk)^T = Xq (Wq Wk^T) Xkv^T = Xq M Xkv^T
with M = Wq Wk^T precomputed once on host (weights-only). This removes the
K-projection (2.1 GF/core) and the K^T AllGather of the previous design.
Per-core FLOPs: 12.88 GF -> 164 us PE floor at 78.6 TF/s bf16.

Nonzero bq/bk are handled by the augmented form (rare path, na=9 tiles):
  scores = [Xq 1] [[M, Wq bk],[(Wk bq)^T, bq.bk]] [Xkv 1]^T
zero-padded from 1025 to 1152 rows/cols. Mask and bv keep their own paths
(mask: pre-scaled add before exp; bv: out accumulates unnormalized and bias
enters as sums[q]*bv so the final 1/sums scale leaves exactly +bv).

Math per core (all matmuls bf16, fp32 PSUM accumulation):
  PT[k,q]   = M-blocks(lhsT) . XqT(rhs)           P = Xq M
  sT[s,q]   = XkvT-tiles(lhsT) . PT(rhs)          scores^T
  eT[s,q]   = exp(sT / sqrt(dk) [+ maskT])        ScalarE, PSUM->SBUF bf16
  sums      = DVE adds over eT s-tiles, then ones-matmul + reciprocal
  HT[v,q]   = Xkv-tiles(lhsT) . eT(rhs)           H = probs_unnorm @ Xkv
  out[q,dv] = HT-tiles(lhsT) . Wv(rhs)  (+bv)     out = H @ Wv
  out      *= 1/sums  (per-partition scale on ScalarE, fused with PSUM->SBUF)

PE group order (zero inter-stage stalls by construction: each stage's first
group depends only on work finished many groups earlier):
  A(c=0..2, i)  ->  B(n=0, m=0..15), B(n=1, m)  ->  D(n=0, j), D(n=1, j)
  ->  E(p=0..3), E(p=4..7); sums emitted inside E p=0.
Stage A runs chunk widths [256,256,512]: the first groups need only
m-block 0 + a 512KB lead of XqT, shrinking the startup-critical DMA set.

DMA design (from v1 trace analysis): aggregate startup bandwidth scales with
in-flight dma_starts; keep payloads ~128-512KB, fan out across sync, gpsimd
and vector queues (scalar gets only 2 issues so its first activation isn't
delayed; vector is free until the sacc chain at ~85us). All inputs are
host-packed into SBUF tile layouts so every dma_start is fully contiguous.
"""

import os
import numpy as np
import ml_dtypes

B, S, D = 4, 2048, 1024
N_CORES = 8
QL = S // 2  # query rows per core (1024)
NT_S = S // 128  # 16 s-tiles
BF16 = ml_dtypes.bfloat16

_cache: dict = {}

AQCH = [512, 512]  # stage-A chunk widths (sum = QL)
AOFF = [0, 512]


def _build(na: int, with_mask: bool, with_bv: bool, ps_bufs: int = 5):
    import concourse.bass as bass
    import concourse.mybir as mybir
    import concourse.tile as tile
    from concourse import bacc

    fp32 = mybir.dt.float32
    bf16 = mybir.dt.bfloat16

    nc = bacc.Bacc("TRN2", target_bir_lowering=False, debug=False,
                   num_devices=N_CORES)

    KA = na * 128          # augmented d_model/d_key (1024 or 1152)
    NQ = QL // 512         # 2 query chunks of 512
    NV = D // 128          # 8 v-blocks
    SCALE = 1.0 / float(np.sqrt(D))

    m_d = nc.dram_tensor("m", (128, na * KA), bf16, kind="ExternalInput")
    xq_d = nc.dram_tensor("xqt", (128, na * QL), bf16, kind="ExternalInput")
    kt_d = nc.dram_tensor("xkvt", (128, NT_S * KA), bf16, kind="ExternalInput")
    xkv_d = nc.dram_tensor("xkv", (128, NV * S), bf16, kind="ExternalInput")
    wv_d = nc.dram_tensor("wv", (128, NV * D), bf16, kind="ExternalInput")
    if with_bv:
        bv_d = nc.dram_tensor("bv", (1, D), bf16, kind="ExternalInput")
    if with_mask:
        maskt_d = nc.dram_tensor("maskt", (S, QL), bf16, kind="ExternalInput")
    out_d = nc.dram_tensor("out", (QL, D), bf16, kind="ExternalOutput")

    with tile.TileContext(nc) as tc:
        with (
            tc.tile_pool(name="cons", bufs=1) as cons,
            tc.tile_pool(name="mp", bufs=1) as mp,
            tc.tile_pool(name="xqp", bufs=1) as xqp,
            tc.tile_pool(name="ktp", bufs=1) as ktp,
            tc.tile_pool(name="ptp", bufs=1) as ptp,
            tc.tile_pool(name="etp", bufs=1) as etp,
            tc.tile_pool(name="xkvp", bufs=1) as xkvp,
            tc.tile_pool(name="wvp", bufs=1) as wvp,
            tc.tile_pool(name="htp", bufs=1) as htp,
            tc.tile_pool(name="outp", bufs=2) as outp,
            tc.tile_pool(name="maskp", bufs=2) as maskp,
            tc.tile_pool(name="ps", bufs=ps_bufs,
                         space=bass.MemorySpace.PSUM) as psp,
            tc.tile_pool(name="pss", bufs=1, space=bass.MemorySpace.PSUM) as pssp,
            tc.tile_pool(name="pst", bufs=1, space=bass.MemorySpace.PSUM) as pstp,
        ):
            sy, sc, gp, ve = nc.sync, nc.scalar, nc.gpsimd, nc.vector

            # ---- SBUF tiles ----
            m_blks = [mp.tile([128, KA], bf16, tag=f"m{i}", name=f"m{i}")
                      for i in range(na)]
            xq_chs = [xqp.tile([128, na, AQCH[c]], bf16, tag=f"xq{c}",
                               name=f"xq{c}") for c in range(len(AQCH))]
            kt_sb = ktp.tile([128, NT_S, KA], bf16, tag="kt")
            pt_sb = ptp.tile([128, na, QL], bf16, tag="pt")
            et_sb = etp.tile([128, NT_S, QL], bf16, tag="et")
            xkv_sb = xkvp.tile([128, NV, NT_S, 128], bf16, tag="xkv")
            wv_sb = wvp.tile([128, NV, D], bf16, tag="wv")
            ht_sb = htp.tile([128, NV, QL], bf16, tag="ht")

            # ---- DMA issue schedule (priority order per engine) ----
            def ld_m(eng, i, lo=0, hi=None):
                hi = KA if hi is None else hi
                eng.dma_start(m_blks[i][:, lo:hi],
                              m_d.ap()[:, i * KA + lo:i * KA + hi])

            def ld_xq(eng, c, jlo, jhi):
                w = AQCH[c]
                base = na * AOFF[c]
                eng.dma_start(xq_chs[c][:, jlo:jhi, :],
                              xq_d.ap()[:, base + jlo * w:base + jhi * w])

            def ld_kt(eng, mlo, mhi):
                eng.dma_start(kt_sb[:, mlo:mhi, :],
                              kt_d.ap()[:, mlo * KA:mhi * KA])

            def ld_xkv(eng, j):
                eng.dma_start(xkv_sb[:, j, :, :],
                              xkv_d.ap()[:, j * S:(j + 1) * S])

            def ld_wv(eng, jlo, jhi):
                eng.dma_start(wv_sb[:, jlo:jhi, :],
                              wv_d.ap()[:, jlo * D:jhi * D])

            # startup-critical wave: first A group needs m0 + xq chunk 0;
            # m[i] is then consumed every ~1.7us. tiny dedicated leads put
            # the first matmul earlier (cold rings run ~100GB/s), the rest
            # follows right behind.
            sy.dma_start(m_blks[0][:, 0:128], m_d.ap()[:, 0:128])
            gp.dma_start(xq_chs[0][:, 0:1, :], xq_d.ap()[:, 0:AQCH[0]])
            sc.dma_start(m_blks[0][:, 128:512], m_d.ap()[:, 128:512])
            sy.dma_start(m_blks[0][:, 512:KA], m_d.ap()[:, 512:KA])
            gp.dma_start(xq_chs[0][:, 1:4, :],
                         xq_d.ap()[:, AQCH[0]:4 * AQCH[0]])
            sc.dma_start(xq_chs[0][:, 4:na, :],
                         xq_d.ap()[:, 4 * AQCH[0]:na * AQCH[0]])
            ld_m(sy, 1)
            ld_m(gp, 2)
            ld_m(sy, 3); ld_m(gp, 4)
            ld_m(sy, 5); ld_m(gp, 6)
            ld_m(sy, 7)
            if na > 8:
                ld_m(gp, 8)

            # stage-B lhsT: 16 s-blocks streamed during B_n0 (from ~25us,
            # one per 1.7us); reused by B_n1.
            for m4 in range(0, NT_S, 4):
                ld_kt(gp, m4, m4 + 2)
                ld_kt(sy, m4 + 2, m4 + 4)

            # xq chunk 1 feeds A_c1 (~52us)
            ld_xq(sy, 1, 0, 4); ld_xq(gp, 1, 4, na)

            # stage-D operands (from ~94us) and stage-E weights (~150us)
            for j in range(NV):
                ld_xkv((sy, gp)[j % 2], j)
            ld_wv(gp, 0, 2); ld_wv(sy, 2, 4)
            ld_wv(gp, 4, 6); ld_wv(sy, 6, 8)

            # ---- constants (tiny, off the critical path) ----
            ones_col = cons.tile([128, 1], fp32, tag="ones_col")
            gp.memset(ones_col[:], 1.0)
            ident1 = cons.tile([1, 1], fp32, tag="ident1")
            gp.memset(ident1[:], 1.0)
            if with_bv:
                bv_sb = cons.tile([1, D], bf16, tag="bv")
                sy.dma_start(bv_sb[:], bv_d.ap()[:])

            # ---- stages A and B, interleaved by q-chunk ----
            # A_c0, B_n0, A_c1, B_n1: spreads the startup-critical DMA set
            # (m + xq chunk 0, then kt streamed during B_n0) across ~50us of
            # compute instead of demanding all of m in stage-A's first 7us.
            def a_chunk(c):
                w, off = AQCH[c], AOFF[c]
                for i in range(na):
                    ps = psp.tile([128, 512], fp32, tag="ps")
                    for j in range(na):
                        nc.tensor.matmul(
                            ps[:, :w], m_blks[i][:, j * 128:(j + 1) * 128],
                            xq_chs[c][:, j, :],
                            start=(j == 0), stop=(j == na - 1))
                    nc.scalar.activation(
                        pt_sb[:, i, off:off + w], ps[:, :w],
                        mybir.ActivationFunctionType.Copy)

            def b_chunk(n):
                for m in range(NT_S):
                    ps = psp.tile([128, 512], fp32, tag="ps")
                    for j in range(na):
                        nc.tensor.matmul(
                            ps[:], kt_sb[:, m, j * 128:(j + 1) * 128],
                            pt_sb[:, j, n * 512:(n + 1) * 512],
                            start=(j == 0), stop=(j == na - 1))
                    if with_mask:
                        mk = maskp.tile([128, 512], bf16, tag="mask")
                        sy.dma_start(
                            mk[:], maskt_d.ap()[m * 128:(m + 1) * 128,
                                                n * 512:(n + 1) * 512])
                        nc.vector.tensor_tensor(
                            ps[:], ps[:], mk[:], mybir.AluOpType.add)
                    nc.scalar.activation(
                        et_sb[:, m, n * 512:(n + 1) * 512], ps[:],
                        mybir.ActivationFunctionType.Exp, scale=SCALE)

            a_chunk(0)
            b_chunk(0)
            a_chunk(1)
            b_chunk(1)

            # softmax denominators on the DVE (PE has no slack; DVE does).
            sacc = cons.tile([128, QL], fp32, tag="sacc")
            nc.vector.tensor_tensor(
                sacc[:], et_sb[:, 0, :], et_sb[:, 1, :], mybir.AluOpType.add)
            for m in range(2, NT_S):
                nc.vector.tensor_tensor(
                    sacc[:], sacc[:], et_sb[:, m, :], mybir.AluOpType.add)

            sums_sb = cons.tile([1, QL], fp32, tag="sums")
            pst = pstp.tile([128, 8], fp32, tag="pst")
            recip_sb = cons.tile([128, 8], fp32, tag="recip")
            if with_bv:
                sums_bf = cons.tile([1, QL], bf16, tag="sums_bf")

            # sums pieces woven between stage-D groups: the pss bank (bufs=1)
            # serializes on its ScalarE drain, so a 3.4us D group between the
            # two sums matmuls hides every handoff.
            def sums_piece(k):
                if k < NQ:
                    pss = pssp.tile([1, 512], fp32, tag="pss")
                    nc.tensor.matmul(
                        pss[:], ones_col[:], sacc[:, k * 512:(k + 1) * 512],
                        start=True, stop=True)
                    nc.scalar.activation(
                        sums_sb[:, k * 512:(k + 1) * 512], pss[:],
                        mybir.ActivationFunctionType.Copy)
                elif k < 4:
                    for p in range(4 * (k - 2), 4 * (k - 1)):
                        nc.tensor.transpose(
                            pst[:, p:p + 1], sums_sb[:, p * 128:(p + 1) * 128],
                            ident1[:])
                elif k == 4:
                    nc.vector.reciprocal(recip_sb[:], pst[:])
                    if with_bv:
                        # out accumulates UNNORMALIZED; bias enters as
                        # sums[q]*bv so the 1/sums scale leaves exactly +bv
                        nc.scalar.activation(
                            sums_bf[:], sums_sb[:],
                            mybir.ActivationFunctionType.Copy)

            # ---- stage D: HT = Xkv-tiles . eT ----
            for n in range(NQ):
                for j in range(NV):
                    ps = psp.tile([128, 512], fp32, tag="ps")
                    for m in range(NT_S):
                        nc.tensor.matmul(
                            ps[:], xkv_sb[:, j, m, :],
                            et_sb[:, m, n * 512:(n + 1) * 512],
                            start=(m == 0), stop=(m == NT_S - 1))
                    nc.scalar.activation(
                        ht_sb[:, j, n * 512:(n + 1) * 512], ps[:],
                        mybir.ActivationFunctionType.Copy)
                    if n == 1:
                        sums_piece(j)

            # ---- stage E: out = HT^T . Wv (+bv), normalized, bf16 out ----
            # p=0: matmuls first, then the sums block, then normalization.
            # The last p runs its final 512 cols as two 256-col groups with
            # act+store flushed per group, so only one short act+DMA chain
            # trails the last matmul.
            for p in range(8):
                out_sb = outp.tile([128, D], bf16, tag="outsb")
                widths = ([(0, 512), (512, 512)] if p < 7
                          else [(0, 512), (512, 256), (768, 256)])
                pending = []

                def flush(p=p, out_sb=out_sb, pending=pending):
                    for ps, lo, w in pending:
                        if with_bv:
                            nc.tensor.matmul(
                                ps[:, :w], sums_bf[:, p * 128:(p + 1) * 128],
                                bv_sb[:, lo:lo + w],
                                start=False, stop=True)
                        nc.scalar.activation(
                            out_sb[:, lo:lo + w], ps[:, :w],
                            mybir.ActivationFunctionType.Copy,
                            scale=recip_sb[:, p:p + 1])
                        # p=7 stores all go on sync: a store issued on the
                        # scalar stream would delay the final activations
                        eng = (nc.sync if p == 7 or (p + lo // 512) % 2 == 0
                               else nc.scalar)
                        eng.dma_start(
                            out_d.ap()[p * 128:(p + 1) * 128, lo:lo + w],
                            out_sb[:, lo:lo + w])
                    pending.clear()

                for lo, w in widths:
                    ps = psp.tile([128, 512], fp32, tag="ps")
                    for j in range(NV):
                        nc.tensor.matmul(
                            ps[:, :w], ht_sb[:, j, p * 128:(p + 1) * 128],
                            wv_sb[:, j, lo:lo + w],
                            start=(j == 0),
                            stop=(j == NV - 1 and not with_bv))
                    pending.append((ps, lo, w))
                    flush()

    nc.compile()
    return nc


def _get_nc(flags):
    if flags not in _cache:
        _cache[flags] = _build(*flags)
    return _cache[flags]


def _pack_blocks(X, nj, ni):
    # X [nj*128, ni*128] -> [128, ni*nj*128]: out[p, (i*nj+j)*128+c]
    # = X[j*128+p, i*128+c] (i-block-major SBUF tile layout, each i-block
    # one fully-contiguous DMA)
    return np.ascontiguousarray(
        X.reshape(nj, 128, ni, 128).transpose(1, 2, 0, 3)
        .reshape(128, ni * nj * 128))


def _pack_x(xt, widths, na):
    # xt [na*128, L] -> [128, sum_c na*w_c]: per col-chunk c,
    # block[p, j*w + t] = xt[j*128+p, off+t]
    blocks = []
    off = 0
    for w in widths:
        blk = xt[:, off:off + w].reshape(na, 128, w).transpose(1, 0, 2)
        blocks.append(blk.reshape(128, na * w))
        off += w
    return np.ascontiguousarray(np.concatenate(blocks, axis=1))


def _prep_in_maps(query_input, keyvalue_input, mask, Wq, bq, Wk, bk, Wv, bv):
    qi = np.asarray(query_input, np.float32)
    kv = np.asarray(keyvalue_input, np.float32)
    mask = np.asarray(mask, np.float32)
    Wq = np.asarray(Wq, np.float32)
    Wk = np.asarray(Wk, np.float32)
    Wvb = np.asarray(Wv, np.float32).astype(BF16)
    bq = np.asarray(bq, np.float32)
    bk = np.asarray(bk, np.float32)
    bv = np.asarray(bv, np.float32)

    with_mask = bool(np.any(mask != 0.0))
    with_bias = bool(np.any(bq != 0.0) or np.any(bk != 0.0))
    with_bv = bool(np.any(bv != 0.0))
    na = 9 if with_bias else 8
    KA = na * 128
    flags = (na, with_mask, with_bv)

    # M = Wq Wk^T (weights-only fold), augmented with q/k biases if nonzero
    M = np.zeros((KA, KA), np.float32)
    M[:D, :D] = Wq @ Wk.T
    if with_bias:
        M[:D, D] = Wq @ bk
        M[D, :D] = Wk @ bq
        M[D, D] = float(bq @ bk)
    M_p = _pack_blocks(M.astype(BF16), na, na)
    wv_p = np.ascontiguousarray(
        Wvb.reshape(8, 128, D).transpose(1, 0, 2).reshape(128, 8 * D))

    in_maps = []
    for c in range(N_CORES):
        b, h = c // 2, c % 2
        xq = qi[b, h * QL:(h + 1) * QL, :]          # [QL, D] fp32
        xkv = kv[b]                                  # [S, D] fp32
        xqt = np.zeros((KA, QL), np.float32)
        xqt[:D] = xq.T
        xkvt = np.zeros((KA, S), np.float32)
        xkvt[:D] = xkv.T
        if with_bias:
            xqt[D] = 1.0
            xkvt[D] = 1.0
        m_ = {
            "m": M_p,
            "xqt": _pack_x(xqt.astype(BF16), AQCH, na),
            "xkvt": _pack_blocks(xkvt.astype(BF16), na, NT_S),
            "xkv": _pack_blocks(xkv.astype(BF16), NT_S, 8),
            "wv": wv_p,
        }
        if with_bv:
            m_["bv"] = bv.astype(BF16).reshape(1, D)
        if with_mask:
            mt = mask[b, h * QL:(h + 1) * QL, :].T * np.float32(np.sqrt(D))
            m_["maskt"] = np.ascontiguousarray(mt.astype(BF16))
        in_maps.append(m_)
    return flags, in_maps


def _ensure_axon_hooks_stub():
    # bass_utils imports antenv.axon_hooks when tracing is requested; the
    # module is absent on some images, so register a no-op stub if needed.
    import sys, types
    try:
        import antenv.axon_hooks  # noqa: F401
    except ImportError:
        stub = types.ModuleType("antenv.axon_hooks")
        stub._hook = None
        stub.set_axon_ntff_profile_hook = (
            lambda h: setattr(stub, "_hook", h))
        stub.get_axon_ntff_profile_hook = lambda: stub._hook
        sys.modules["antenv.axon_hooks"] = stub
        try:
            import antenv
            antenv.axon_hooks = stub
        except ImportError:
            pass


def _run(inputs, trace=False, **kw):
    _ensure_axon_hooks_stub()
    from concourse import bass_utils
    ps_bufs = int(os.environ.get("KERNEL_PSBUFS", "6"))
    flags, in_maps = _prep_in_maps(**inputs)
    nc = _get_nc(flags + (ps_bufs,))
    res = bass_utils.run_bass_kernel_spmd(
        nc, in_maps, core_ids=list(range(N_CORES)), trace=trace, **kw)
    out = np.empty((B, S, D), np.float32)
    for c in range(N_CORES):
        b, h = c // 2, c % 2
        out[b, h * QL:(h + 1) * QL, :] = np.asarray(
            res.results[c]["out"], dtype=np.float32)
    return out, res


def kernel(**inputs) -> np.ndarray:
    out, _ = _run(inputs, trace=False)
    return out
